# revision 3
# baseline (speedup 1.0000x reference)
"""Trainium2 Bass kernel for nn_B_188978561578.

reference: y successive elementwise float32 divisions of x by 10,
x shape (32, 2048, 2048) fp32. Pure elementwise, memory-bound.

Strategy: data-parallel shard along batch dim across 8 NeuronCores
(4 batches/core = 64 MiB/core). Each core streams its shard through
SBUF in [128, 8192] fp32 tiles (4 MiB DMAs -> near line rate), applies
one fused scalar multiply by 10^-y on the Vector engine, and streams
back out. Loads issue on the SP HWDGE ring, stores on the ACT HWDGE
ring so they never head-of-line block each other.
"""

import numpy as np

N_CORES = 8
B, H, W = 32, 2048, 2048          # full input shape
B_PER_CORE = B // N_CORES         # 4
P = 128                           # SBUF partitions
F = 16384                         # free elems per tile (64 KiB/partition)
ELEMS_PER_CORE = B_PER_CORE * H * W
TILES = ELEMS_PER_CORE // (P * F)  # 8

_compiled_cache: dict[float, object] = {}


def _build(scale: float):
    import concourse.tile as tile
    import concourse.mybir as mybir
    from concourse import bacc

    nc = bacc.Bacc("TRN2", target_bir_lowering=False, debug=False)
    x_in = nc.dram_tensor("x", [TILES, P, F], mybir.dt.float32, kind="ExternalInput")
    out = nc.dram_tensor("out", [TILES, P, F], mybir.dt.float32, kind="ExternalOutput")
    H2 = F // 2
    with tile.TileContext(nc) as tc:
        with tc.tile_pool(name="sbuf", bufs=3) as pool:
            for t in range(TILES):
                tl = pool.tile([P, F], mybir.dt.float32)
                nc.sync.dma_start(tl[:, :H2], x_in[t, :, :H2])
                nc.sync.dma_start(tl[:, H2:], x_in[t, :, H2:])
                nc.vector.tensor_scalar_mul(tl[:], tl[:], scale)
                nc.scalar.dma_start(out[t, :, :H2], tl[:, :H2])
                nc.scalar.dma_start(out[t, :, H2:], tl[:, H2:])
    nc.compile()
    return nc


def _get_compiled(scale: float):
    if scale not in _compiled_cache:
        _compiled_cache[scale] = _build(scale)
    return _compiled_cache[scale]


def kernel(x: np.ndarray, y) -> np.ndarray:
    from concourse.bass_utils import run_bass_kernel_spmd

    yi = int(np.asarray(y).item())
    # Single multiply by fp32(10^-y): within ~8 ulps of the reference's
    # y-step rounded division chain.
    scale = float(np.float32(np.float64(10.0) ** (-yi)))

    x = np.ascontiguousarray(np.asarray(x, dtype=np.float32))
    nc = _get_compiled(scale)

    shards = [
        x[c * B_PER_CORE:(c + 1) * B_PER_CORE].reshape(TILES, P, F)
        for c in range(N_CORES)
    ]
    res = run_bass_kernel_spmd(
        nc, [{"x": s} for s in shards], core_ids=list(range(N_CORES))
    )
    return np.concatenate(
        [r["out"].reshape(B_PER_CORE, H, W) for r in res.results], axis=0
    )
